# revision 33
# baseline (speedup 1.0000x reference)
"""Trainium2 Bass kernel for the ChunkedSIEVE model (segment_reduce).

Math (see reference):
  x[b,v,:]  = tanh(feat[b,v,:] @ W_feat + b_feat + pos[b,v]*1e-6 * w_pos)
              + gene_table[gene_ids[b,v]]
  emb[b]    = mean_v x[b,v,:]                      (mask is all ones)
  scores[b] = tanh(emb @ W_att1 + b_att1) @ W_att2 (+ b_att2, cancels in softmax)
  per-sample (8 contiguous chunks) softmax over scores -> w
  out[s]    = sum_b w[b] * (emb[b] @ W_cls) + b_cls

Strategy: data-parallel over chunks, 256 chunks (32 samples) per core.
Everything stays in [D x chunk] layout:
  - PE computes z = [W_feat; w_pos]^T @ featT in bf16 (K=65: 64 features +
    the scaled-position row appended on the host).
  - ACT applies tanh with the per-partition b_feat bias straight out of
    PSUM.  The drain of z out of PSUM through ACT (1 elem/cycle/lane) is
    the kernel's critical path, so PSUM is organised as two 3-bank
    ping-pong tiles and each ACTIVATE covers 1536 columns (6 chunks) to
    amortise the per-instruction overhead.
  - The V-sum of the tanh term runs as a pairwise bf16 tensor_tensor tree
    on DVE (2x mode) + a short fp32 reduce tail, per 24-chunk group.
  - The gene-table term is a segment-sum over the table, computed as a
    matmul: gsum[d,c] = sum_g gene[g,d] * hist[g,c], where hist is the
    per-chunk gene-id histogram (fp8, exact small counts) built on the
    host.  This replaces the row-gather entirely.
  - A tiny pair of fp32 matmuls projects t1 by [W_att1 | W_cls]/V, then
    the per-sample softmax runs in a [1, BC] layout.
"""

import functools
import os
import sys

import numpy as np

for _p in ("/opt/trn_rl_repo",):
    if _p not in sys.path and os.path.isdir(_p):
        sys.path.insert(0, _p)

import ml_dtypes  # noqa: E402

import concourse.bass as bass  # noqa: E402
import concourse.tile as tile  # noqa: E402
from concourse import bacc, mybir  # noqa: E402
from concourse.bass_utils import run_bass_kernel_spmd  # noqa: E402
from contextlib import ExitStack  # noqa: E402

F32 = mybir.dt.float32
BF16 = mybir.dt.bfloat16
FP8 = mybir.dt.float8e4
AF = mybir.ActivationFunctionType
ALU = mybir.AluOpType
AX = mybir.AxisListType

B, V, F, D, G, S = 2048, 256, 64, 256, 20000, 256
POS_SCALE = 1e-6
NCORES = 8
BC = B // NCORES          # 256 chunks per core
RC = BC * V               # 65536 rows per core
SC = S // NCORES          # 32 samples per core
K8 = B // S               # 8 chunks per sample
KIN = F + 1               # 65 = features + position row

ST_CH = 6                 # chunks per supertile (3 PSUM banks)
ST_ROWS = ST_CH * V       # 1536
NST = (BC + ST_CH - 1) // ST_CH   # 43 supertiles (last has 4 chunks)
GRP_CH = 24               # chunks per DVE-tree group (4 supertiles)
NGRP = (BC + GRP_CH - 1) // GRP_CH  # 11 groups (last has 16 chunks)

GKT = (G + 127) // 128    # 157 gene k-tiles
GPAD = GKT * 128          # 20096
GCH = 8                   # gene k-tiles per DMA chunk (one trigger each)
HPS = 6                   # hist k-tiles interleaved per supertile
GENE_SCALE = 64.0         # fp8 range scale for the gene table

# supertiles whose h0 half drains through DVE as a fitted cubic
# (tanh(x) ~ c0*x + c1*x^3) instead of through ACT: the last supertile of
# each of groups 0..7.  Balances the PSUM-drain between ACT and DVE.
CUBIC_S = frozenset(4 * g + 3 for g in range(8))
TAIL_C0 = 192             # chunks >= this get per-supertile trees + projs


def _chunks_of_st(s):
    c0 = s * ST_CH
    return c0, min(BC, c0 + ST_CH) - c0


def _chunks_of_grp(g):
    c0 = g * GRP_CH
    return c0, min(BC, c0 + GRP_CH) - c0


def _emit(nc, tc, featT, geneT, histT, w65, cblob, zlb, out):
    ctx = ExitStack()
    with ctx:
        const = ctx.enter_context(tc.tile_pool(name="const", bufs=1))
        acc = ctx.enter_context(tc.tile_pool(name="acc", bufs=1))
        feat_p = ctx.enter_context(tc.tile_pool(name="feat", bufs=4))
        gene_p = ctx.enter_context(tc.tile_pool(name="gene", bufs=10))
        hist_p = ctx.enter_context(tc.tile_pool(name="hist", bufs=10))
        xg_p = ctx.enter_context(tc.tile_pool(name="xg", bufs=4))
        tree_p = ctx.enter_context(tc.tile_pool(name="tree", bufs=1))
        zpool = ctx.enter_context(tc.tile_pool(name="zp", bufs=2, space="PSUM"))
        gsum_p = ctx.enter_context(tc.tile_pool(name="gs", bufs=1, space="PSUM"))
        small = ctx.enter_context(tc.tile_pool(name="small", bufs=1))

        # ---- constants: w65 + one packed f32 blob, then feat prefetches ----
        w65_t = const.tile([KIN, D], BF16)
        nc.sync.dma_start(w65_t[:, :], w65[:, :])
        cblob_t = const.tile([128, 136], F32)
        nc.sync.dma_start(cblob_t[:, :], cblob[:, :])
        bf_t = cblob_t[:, 0:2]
        psc_t0 = cblob_t[:, 2:2 + KIN]
        psc_t1 = cblob_t[:, 2 + KIN:2 + 2 * KIN]
        batt1_t = cblob_t[0:64, 132:133]
        watt2_t = cblob_t[0:64, 133:134]
        bcls_t = cblob_t[0:1, 134:135]
        cub_c1 = cblob_t[:, 135:136]
        ft_tiles = {}

        def emit_ft_dma(s, engine=None):
            if s >= NST or s in ft_tiles:
                return
            ft = feat_p.tile([KIN, ST_ROWS], BF16, tag="ft", name="ft")
            c0 = s * ST_ROWS
            c1 = min(RC, c0 + ST_ROWS)
            (engine or nc.sync).dma_start(ft[:, 0:c1 - c0], featT[:, c0:c1])
            ft_tiles[s] = ft

        # the first two feature tiles (and the small zlin blob) ride the
        # (idle) Act-HWDGE ring so the startup transfers run in parallel
        emit_ft_dma(0, nc.scalar)
        emit_ft_dma(1, nc.scalar)
        zlb_t = const.tile([KIN + 1, 384], F32)
        nc.scalar.dma_start(zlb_t[:, :], zlb[:, :])
        emit_ft_dma(2)
        emit_ft_dma(3)

        # warm the ACT function table while the first tiles stream in
        # (uninitialized read is harmless: tanh output is never consumed)
        dummy = small.tile([1, 8], F32)
        nc.scalar.activation(dummy[:, :], dummy[:, :], AF.Tanh)

        # both D-halves side by side: [:, 0:BC] is half 0, [:, BC:2BC] half 1
        t1h = acc.tile([128, 2 * BC], F32)
        t1v = t1h.rearrange("p (h c) -> p h c", h=2)
        # c0 * sum_v z for the cubic-drained chunks (h0 only)
        zl = acc.tile([128, BC], F32)
        # one PSUM bank holds both gsum halves side by side; a second holds
        # the projection outputs (h = [W_att1|W_cls]^T emb) and the scores
        gsum_b = gsum_p.tile([128, 2 * BC], F32, tag="gs", name="gsum_b")
        gsum = [gsum_b[:, h * BC:(h + 1) * BC] for h in range(2)]
        psHS = gsum_p.tile([128, 2 * BC], F32, tag="ph", name="psHS")
        xg = [None]
        # tree scratch (shared across groups; DVE is serial anyway)
        yA = tree_p.tile([128, 2 * GRP_CH * 128], BF16)
        yB = tree_p.tile([128, 2 * GRP_CH * 64], BF16)
        # cubic-drain scratch
        xs_t = tree_p.tile([128, ST_ROWS], BF16)
        yy_t = tree_p.tile([128, ST_ROWS], BF16)

        # DRAM views of gene table / hist: host supplies [128, kt, n]
        # partition-major layout (long contiguous per-partition lines)
        geneT_v = geneT.rearrange("p (t d) -> p t d", d=D)
        histT_v = histT.rearrange("p (t c) -> p t c", c=BC)

        # interleave schedule for the gene-histogram matmuls
        kt_next = [0]
        chunk_tiles = {}

        def emit_chunk_dma(ck):
            if ck * GCH >= GKT or ck in chunk_tiles:
                return
            k0 = ck * GCH
            cs = min(GCH, GKT - k0)
            gt = gene_p.tile([128, GCH, D], FP8, tag="gt", name="gt")
            ht = hist_p.tile([128, GCH, BC], FP8, tag="ht", name="ht")
            nc.sync.dma_start(gt[:, 0:cs, :], geneT_v[:, k0:k0 + cs, :])
            nc.sync.dma_start(ht[:, 0:cs, :], histT_v[:, k0:k0 + cs, :])
            chunk_tiles[ck] = (gt, ht)

        def emit_hist_upto(limit):
            while kt_next[0] < min(limit, GKT):
                kt = kt_next[0]
                emit_chunk_dma(kt // GCH)
                gt, ht = chunk_tiles[kt // GCH]
                loc = kt % GCH
                for h in range(2):
                    nc.tensor.matmul(
                        gsum[h],
                        gt[:, loc, h * 128:(h + 1) * 128],
                        ht[:, loc, :],
                        start=(kt == 0),
                        stop=(kt == GKT - 1),
                    )
                kt_next[0] += 1

        def emit_tree(g, c0, c1):
            """V-sum tree (both halves at once) for local chunks [c0, c1) of
            the current xg tile."""
            nch = c1 - c0
            gc0, gn = _chunks_of_grp(g)
            n2 = 2 * nch
            xv = xg[0].rearrange("p (h c v) -> p h c v", h=2,
                                 v=V)[:, :, c0:c1, :]
            nc.vector.tensor_add(
                yA[:, 0:n2 * 128].rearrange("p (c v) -> p c v", v=128),
                xv[:, :, :, 0:128], xv[:, :, :, 128:256])
            a1 = yA[:, 0:n2 * 128].rearrange("p (c v) -> p c v", v=128)
            nc.vector.tensor_add(
                yB[:, 0:n2 * 64].rearrange("p (c v) -> p c v", v=64),
                a1[:, :, 0:64], a1[:, :, 64:128])
            a2 = yB[:, 0:n2 * 64].rearrange("p (c v) -> p c v", v=64)
            nc.vector.tensor_add(
                yA[:, 0:n2 * 32].rearrange("p (c v) -> p c v", v=32),
                a2[:, :, 0:32], a2[:, :, 32:64])
            a3 = yA[:, 0:n2 * 32].rearrange("p (c v) -> p c v", v=32)
            nc.vector.tensor_add(
                yB[:, 0:n2 * 16].rearrange("p (c v) -> p c v", v=16),
                a3[:, :, 0:16], a3[:, :, 16:32])
            a4 = yB[:, 0:n2 * 16].rearrange("p (c v) -> p c v", v=16)
            nc.vector.tensor_add(
                yA[:, 0:n2 * 8].rearrange("p (c v) -> p c v", v=8),
                a4[:, :, 0:8], a4[:, :, 8:16])
            nc.vector.reduce_sum(
                t1v[:, :, gc0 + c0:gc0 + c1],
                yA[:, 0:n2 * 8].rearrange("p (h c v) -> p h c v", h=2, v=8),
                axis=AX.X,
            )

        def emit_proj(c0, c1):
            """Add the (rescaled) gene segment-sum and project chunks
            [c0, c1).  Deferred past the last hist matmul."""
            gs = slice(c0, c1)
            for h in range(2):
                nc.vector.scalar_tensor_tensor(
                    t1h[:, h * BC + c0:h * BC + c1], gsum[h][:, gs],
                    1.0 / GENE_SCALE,
                    t1h[:, h * BC + c0:h * BC + c1],
                    op0=ALU.mult, op1=ALU.add)
            hv = psHS[0:KIN, gs]
            nc.tensor.matmul(hv, psc_t0[:, :], t1h[:, c0:c1],
                             start=True, stop=False)
            nc.tensor.matmul(hv, psc_t1[:, :], t1h[:, BC + c0:BC + c1],
                             start=False, stop=True)

        # ---- main loop ----
        projected = [False] * NGRP
        for ck in range(3):
            emit_chunk_dma(ck)

        for s in range(NST):
            c0, nch = _chunks_of_st(s)
            g = c0 // GRP_CH
            gc0, gn = _chunks_of_grp(g)
            if c0 == gc0:
                xg[0] = xg_p.tile([128, 2 * GRP_CH * V], BF16, tag="xg",
                                  name="xg")
            ft = ft_tiles.pop(s)
            emit_ft_dma(s + 3)
            off = (c0 - gc0) * V
            rows = nch * V
            for h in range(2):
                zp = zpool.tile([128, ST_ROWS], F32, tag="ps", name="zp")
                for q in range(rows // 512):
                    nc.tensor.matmul(
                        zp[:, q * 512:(q + 1) * 512],
                        w65_t[:, h * 128:(h + 1) * 128],
                        ft[:, q * 512:(q + 1) * 512],
                        start=True,
                        stop=True,
                    )
                dst = xg[0][:, h * GRP_CH * V + off:h * GRP_CH * V + off
                            + rows]
                if h == 0 and s in CUBIC_S:
                    # drain through DVE: x (bf16), then x^3 into the xg slot
                    nc.vector.tensor_copy(xs_t[:, 0:rows], zp[:, 0:rows])
                    nc.vector.tensor_mul(yy_t[:, 0:rows], xs_t[:, 0:rows],
                                         xs_t[:, 0:rows])
                    nc.vector.tensor_mul(dst, yy_t[:, 0:rows],
                                         xs_t[:, 0:rows])
                else:
                    nc.scalar.activation(dst, zp[:, 0:rows], AF.Tanh,
                                         bias=bf_t[:, h:h + 1])
            if s == 1:
                # zlin = (c0-scaled W) @ per-chunk feature sums, staged
                # through the gsum bank (hist matmuls only start at s=4)
                nc.tensor.matmul(gsum[0], zlb_t[:, 0:128],
                                 zlb_t[:, 128:128 + BC],
                                 start=True, stop=True)
                nc.vector.tensor_copy(zl[:, :], gsum[0])
            if 1 <= s <= 3:
                emit_chunk_dma(s - 1)
            if s >= 4:
                pf = min(HPS * (s - 3) // GCH + 2, (GKT - 1) // GCH)
                for ck in range(pf + 1):
                    emit_chunk_dma(ck)
                emit_hist_upto(HPS * (s - 3))
            # tail region: tree per supertile to shorten the tail; projs at
            # group granularity except for the very last group
            if c0 >= TAIL_C0:
                emit_tree(g, c0 - gc0, c0 - gc0 + nch)
                if g == NGRP - 1:
                    emit_proj(c0, c0 + nch)
                elif c0 + nch == gc0 + gn:
                    emit_proj(gc0, gc0 + gn)
            elif c0 + nch == gc0 + gn:
                emit_tree(g, 0, gn)
                if s in CUBIC_S:
                    # t1(h0, cubic chunks) = c1 * sum(x^3) + c0 * sum(x)
                    nc.vector.scalar_tensor_tensor(
                        t1h[:, c0:c0 + nch], t1h[:, c0:c0 + nch], cub_c1,
                        zl[:, c0:c0 + nch], op0=ALU.mult, op1=ALU.add)
            # project once the hist matmuls are flushed
            if kt_next[0] >= GKT:
                done_g = (c0 + nch) // GRP_CH  # groups fully treed
                for gg in range(min(done_g, TAIL_C0 // GRP_CH)):
                    if not projected[gg]:
                        gg0, ggn = _chunks_of_grp(gg)
                        emit_proj(gg0, gg0 + ggn)
                        projected[gg] = True

        # ---- attention scores + classifier from the projections ----
        u_t = small.tile([64, BC], F32)
        nc.scalar.activation(u_t[:, :], psHS[0:64, 0:BC], AF.Tanh,
                             bias=batt1_t[:, :])
        psS = psHS[0:1, BC:2 * BC]
        nc.tensor.matmul(psS, watt2_t[:, :], u_t[:, :],
                         start=True, stop=True)

        # ---- per-sample softmax over 8 chunks, all in [1, BC] layout.
        # Scores are bounded (|s| <= ||W_att2||_1 + |b|), so exp() without
        # the max-subtraction is safe in fp32 and softmax is shift-invariant.
        ew = small.tile([1, 2 * BC], F32)
        nc.scalar.activation(ew[:, 0:BC], psS, AF.Exp)
        # wa = (emb @ W_cls / V + b_cls) * e, read straight out of PSUM
        nc.vector.scalar_tensor_tensor(
            ew[:, BC:2 * BC], psHS[64:65, 0:BC], bcls_t[0:1, 0:1],
            ew[:, 0:BC], op0=ALU.add, op1=ALU.mult)
        sums = small.tile([1, 2 * SC], F32)
        nc.vector.reduce_sum(sums[:, :],
                             ew.rearrange("p (q k) -> p q k", k=K8), axis=AX.X)
        rec = small.tile([1, SC], F32)
        nc.vector.reciprocal(rec[:, :], sums[:, 0:SC])
        o_t = small.tile([1, SC], F32)
        nc.vector.tensor_mul(o_t[:, :], sums[:, SC:2 * SC], rec[:, :])
        nc.sync.dma_start(out.rearrange("s o -> o s"), o_t[:, :])


@functools.lru_cache(maxsize=1)
def _build():
    nc = bacc.Bacc(
        "TRN2",
        target_bir_lowering=False,
        debug=False,
        enable_asserts=False,
        num_devices=NCORES,
    )
    featT = nc.dram_tensor("featT", [KIN, RC], BF16, kind="ExternalInput")
    geneT = nc.dram_tensor("geneT", [128, GKT * D], FP8, kind="ExternalInput")
    histT = nc.dram_tensor("histT", [128, GKT * BC], FP8, kind="ExternalInput")
    w65 = nc.dram_tensor("w65", [KIN, D], BF16, kind="ExternalInput")
    cblob = nc.dram_tensor("cblob", [128, 136], F32, kind="ExternalInput")
    zlb = nc.dram_tensor("zlb", [KIN + 1, 384], F32, kind="ExternalInput")
    out = nc.dram_tensor("out", [SC, 1], F32, kind="ExternalOutput")
    with tile.TileContext(nc) as tc:
        _emit(nc, tc, featT.ap(), geneT.ap(), histT.ap(), w65.ap(),
              cblob.ap(), zlb.ap(), out.ap())
    nc.compile()
    return nc


def _prep_inputs(features, positions, gene_ids, mask, original_sample_indices,
                 W_feat, b_feat, gene_table, w_pos,
                 W_att1, b_att1, W_att2, b_att2, W_cls, b_cls):
    features = np.asarray(features, np.float32)
    positions = np.asarray(positions)
    gene_ids = np.asarray(gene_ids)
    BF = ml_dtypes.bfloat16
    F8 = ml_dtypes.float8_e4m3fn

    featT_full = np.empty((KIN, B * V), BF)
    featT_full[:F] = features.reshape(B * V, F).T.astype(BF)
    featT_full[F] = (positions.reshape(-1).astype(np.float32)
                     * POS_SCALE).astype(BF)

    gene_f8 = np.zeros((GPAD, D), F8)
    gene_f8[:G] = (np.asarray(gene_table, np.float32) * GENE_SCALE).astype(F8)
    # partition-major layout: [128, GKT, D] -> [128, GKT*D]
    gene_pm = np.ascontiguousarray(
        gene_f8.reshape(GKT, 128, D).transpose(1, 0, 2)).reshape(128, GKT * D)

    ids = gene_ids.reshape(B, V).astype(np.int64)
    chunk_local = (np.arange(B)[:, None] % BC).repeat(V, axis=1)

    w65f = np.concatenate(
        [np.asarray(W_feat, np.float32),
         np.asarray(w_pos, np.float32)[None, :]], axis=0)
    w65v = w65f.astype(BF)
    pscv = np.concatenate([np.asarray(W_att1, np.float32),
                           np.asarray(W_cls, np.float32)], axis=1) / V
    bfeatv = np.asarray(b_feat, np.float32)

    # fit tanh(x) ~ c0*x + c1*x^3 on a subsample of the actual z values
    fsub = features[::8, ::8, :]
    psub = positions[::8, ::8].astype(np.float32) * POS_SCALE
    zsub = (fsub @ np.asarray(W_feat, np.float32) + bfeatv
            + psub[..., None] * np.asarray(w_pos, np.float32))
    zs = zsub.reshape(-1).astype(np.float64)[::3]
    Afit = np.stack([zs, zs ** 3], 1)
    c0f, c1f = np.linalg.lstsq(Afit, np.tanh(zs), rcond=None)[0]

    cblob = np.zeros((128, 136), np.float32)
    cblob[:, 0] = bfeatv[0:128]
    cblob[:, 1] = bfeatv[128:256]
    cblob[:, 2:2 + KIN] = pscv[0:128, :]
    cblob[:, 2 + KIN:2 + 2 * KIN] = pscv[128:256, :]
    cblob[0:64, 132] = np.asarray(b_att1, np.float32)
    cblob[0:64, 133] = np.asarray(W_att2, np.float32)[:, 0]
    cblob[0, 134] = np.asarray(b_cls, np.float32).reshape(-1)[0]
    cblob[:, 135] = c1f
    cblob = np.ascontiguousarray(cblob)

    zstat = np.zeros((KIN + 1, 128), np.float32)
    zstat[:KIN] = w65f[:, 0:128] * c0f
    zstat[KIN] = bfeatv[0:128] * c0f
    posf = positions.astype(np.float32) * POS_SCALE

    in_maps = []
    for c in range(NCORES):
        ids_c = ids[c * BC:(c + 1) * BC].reshape(-1)
        loc_c = chunk_local[c * BC:(c + 1) * BC].reshape(-1)
        hist = np.bincount(ids_c * BC + loc_c,
                           minlength=GPAD * BC).reshape(GPAD, BC)
        hist_pm = np.ascontiguousarray(
            hist.reshape(GKT, 128, BC).transpose(1, 0, 2)
        ).reshape(128, GKT * BC)
        colsum = np.empty((KIN + 1, BC), np.float32)
        colsum[:F] = features[c * BC:(c + 1) * BC].sum(axis=1).T
        colsum[F] = posf[c * BC:(c + 1) * BC].sum(axis=1)
        colsum[KIN] = V
        zlb = np.ascontiguousarray(
            np.concatenate([zstat, colsum], axis=1))
        in_maps.append({
            "featT": np.ascontiguousarray(featT_full[:, c * RC:(c + 1) * RC]),
            "geneT": gene_pm,
            "histT": hist_pm.astype(F8),
            "w65": w65v,
            "cblob": cblob,
            "zlb": zlb,
        })
    return in_maps


def _run(inputs, trace=False, **kw):
    nc = _build()
    in_maps = _prep_inputs(**inputs)
    res = run_bass_kernel_spmd(
        nc, in_maps, core_ids=list(range(NCORES)), trace=trace, **kw)
    outv = np.concatenate(
        [np.asarray(res.results[c]["out"], np.float32) for c in range(NCORES)],
        axis=0)
    return outv, res


def _numpy_fallback(features, positions, gene_ids, mask,
                    original_sample_indices, W_feat, b_feat, gene_table,
                    w_pos, W_att1, b_att1, W_att2, b_att2, W_cls, b_cls):
    features = np.asarray(features, np.float32)
    mask_f = np.asarray(mask, np.float32)
    pos = np.asarray(positions).astype(np.float32) * POS_SCALE
    x = np.tanh(features @ np.asarray(W_feat, np.float32)
                + np.asarray(b_feat, np.float32)
                + pos[..., None] * np.asarray(w_pos, np.float32))
    x = x + np.asarray(gene_table, np.float32)[np.asarray(gene_ids)]
    denom = np.maximum(mask_f.sum(-1, keepdims=True), 1.0)
    emb = (x * mask_f[..., None]).sum(axis=1) / denom
    scores = (np.tanh(emb @ np.asarray(W_att1, np.float32)
                      + np.asarray(b_att1, np.float32))
              @ np.asarray(W_att2, np.float32)
              + np.asarray(b_att2, np.float32))[:, 0]
    seg = np.asarray(original_sample_indices).astype(np.int64)
    smax = np.full(S, -np.inf, np.float32)
    np.maximum.at(smax, seg, scores)
    e = np.exp(scores - smax[seg])
    ssum = np.zeros(S, np.float32)
    np.add.at(ssum, seg, e)
    w = e / ssum[seg]
    agg = np.zeros((S, D), np.float32)
    np.add.at(agg, seg, emb * w[:, None])
    return agg @ np.asarray(W_cls, np.float32) + np.asarray(b_cls, np.float32)


def kernel(**inputs):
    mask = np.asarray(inputs["mask"])
    seg = np.asarray(inputs["original_sample_indices"]).astype(np.int64)
    expected_seg = np.arange(B) // K8
    if not mask.all() or not np.array_equal(seg, expected_seg):
        return _numpy_fallback(**inputs)
    outv, _ = _run(inputs)
    return outv
